# revision 1
# baseline (speedup 1.0000x reference)
"""MiniMaxText01 linear attention on 8 trn2 NeuronCores.

Sharding: core c -> batch b = c//4, head-quad q = c%4 (global heads 4q..4q+3).
Each core runs 2 sequential head-pair passes (2 heads each); out_proj is
row-parallel so each pass emits a partial [S, HID] output; the host sums the
partials per batch.

All on-chip tensors live in transposed [feature, token] layout so every matmul
contraction sits on the partition dim. Matmul operands are fp16 (same 10-bit
mantissa as TF32) with fp32 PSUM accumulation; host pre-casts and pre-tiles x
and the weights so every DMA is a contiguous per-partition run. Tokens are
processed in 512-wide blocks = 2 attention chunks of 256.
"""

import numpy as np

B, S, HID = 2, 4096, 2048
H, D, C = 16, 128, 256
BLK = 512              # token block (2 chunks)
NBLK = S // BLK        # 8 blocks
KO = HID // 128        # 16 contraction subtiles
NCORES = 8
P = 128

_PROG = None


def _build_program():
    import concourse.bacc as bacc
    import concourse.mybir as mybir
    import concourse.tile as tile

    F32 = mybir.dt.float32
    F16 = mybir.dt.float16
    AF = mybir.ActivationFunctionType
    MUL = mybir.AluOpType.mult
    ADD = mybir.AluOpType.add

    nc = bacc.Bacc("TRN2", target_bir_lowering=False, debug=False,
                   num_devices=NCORES)

    # x pre-tiled on host: [blk, quarter, kp, ko', t] (contiguous per piece)
    xT = nc.dram_tensor("xT", [NBLK, 4, P, KO // 4, BLK], F16,
                        kind="ExternalInput")
    # weights pre-tiled on host: [pass, kp, ko, m]
    wq = nc.dram_tensor("wq", [2, P, KO, 256], F16, kind="ExternalInput")
    wk = nc.dram_tensor("wk", [2, P, KO, 256], F16, kind="ExternalInput")
    wv = nc.dram_tensor("wv", [2, P, KO, 256], F16, kind="ExternalInput")
    wg = nc.dram_tensor("wg", [2, P, KO, 256], F16, kind="ExternalInput")
    # w_out pre-tiled on host: [pass, kp, kh, n]
    wo = nc.dram_tensor("wo", [2, P, 2, HID], F16, kind="ExternalInput")
    ddT = nc.dram_tensor("ddT", [4, 2, P, C], F32, kind="ExternalInput")
    qdec = nc.dram_tensor("qdec", [4, P, C], F16, kind="ExternalInput")
    kdec = nc.dram_tensor("kdec", [4, P, C], F16, kind="ExternalInput")
    bdec = nc.dram_tensor("bdec", [P, 4], F32, kind="ExternalInput")
    idn = nc.dram_tensor("idn", [P, P], F16, kind="ExternalInput")
    outs = [nc.dram_tensor(f"out{p}", [S, HID], F32, kind="ExternalOutput")
            for p in range(2)]

    with tile.TileContext(nc) as tc:
        with tc.tile_pool(name="const", bufs=1) as cpool, \
             tc.tile_pool(name="wpool", bufs=2) as wpool, \
             tc.tile_pool(name="mpool", bufs=2) as mpool, \
             tc.tile_pool(name="kvpool", bufs=2) as kvpool, \
             tc.tile_pool(name="xpool", bufs=16) as xpool, \
             tc.tile_pool(name="qkpool", bufs=2) as qkpool, \
             tc.tile_pool(name="apool", bufs=2) as apool, \
             tc.tile_pool(name="opool", bufs=6) as opool, \
             tc.tile_pool(name="pproj", bufs=2, space="PSUM") as pproj, \
             tc.tile_pool(name="ptr", bufs=2, space="PSUM") as ptr, \
             tc.tile_pool(name="psc", bufs=2, space="PSUM") as psc, \
             tc.tile_pool(name="pout", bufs=2, space="PSUM") as pout:

            ident = cpool.tile([P, P], F16)
            nc.sync.dma_start(ident[:], idn.ap())
            bd_sb = cpool.tile([P, 4], F32)
            nc.sync.dma_start(bd_sb[:], bdec.ap())

            def load_pass_consts(p, after_wq=None):
                ws = {}
                for nm, wd in (("wq", wq), ("wk", wk), ("wv", wv), ("wg", wg)):
                    wt = wpool.tile([P, KO, 256], F16, tag=nm)
                    nc.sync.dma_start(wt[:], wd.ap()[p])
                    ws[nm] = wt
                    if nm == "wq" and after_wq is not None:
                        ws["x0"] = after_wq()
                wo_r = wpool.tile([P, 2, HID], F16, tag="wo")
                nc.sync.dma_start(wo_r[:], wo.ap()[p])
                ws["wo"] = wo_r
                dd_sb = mpool.tile([P, 2, 2, C], F32, tag="dd")
                qd_sb = mpool.tile([P, 2, C], F16, tag="qd")
                kd_sb = mpool.tile([P, 2, C], F16, tag="kd")
                for lh in range(2):
                    gh = 2 * p + lh
                    nc.sync.dma_start(dd_sb[:, lh],
                                      ddT.ap()[gh].rearrange("jt kp i -> kp jt i"))
                    nc.sync.dma_start(qd_sb[:, lh, :], qdec.ap()[gh])
                    nc.sync.dma_start(kd_sb[:, lh, :], kdec.ap()[gh])
                ws.update(dd=dd_sb, qd=qd_sb, kd=kd_sb)
                return ws

            def load_x(blk):
                # four quarter-K pieces, each one contiguous 512KB DMA
                xts = []
                for quarter in range(4):
                    xth = xpool.tile([P, KO // 4, BLK], F16, tag="xt")
                    nc.sync.dma_start(xth[:], xT.ap()[blk, quarter])
                    xts.append(xth)
                return xts

            consts = [None, None]
            consts[0] = load_pass_consts(0, after_wq=lambda: load_x(0))

            for p in range(2):
                cs = consts[p]
                wq_r, wk_r, wv_r, wg_r, wo_r = (cs["wq"], cs["wk"], cs["wv"],
                                                cs["wg"], cs["wo"])
                dd_sb, qd_sb, kd_sb = cs["dd"], cs["qd"], cs["kd"]

                kv_sb = kvpool.tile([P, 2, P], F32, tag="kv")
                nc.vector.memset(kv_sb[:], 0.0)

                cur_x = cs.pop("x0", None) or load_x(0)
                for blk in range(NBLK):
                    t0 = blk * BLK
                    # prefetch next block's x before this block's out-DMAs
                    # enter the (in-order) sync engine's queue
                    xts = cur_x
                    if blk + 1 < NBLK:
                        cur_x = load_x(blk + 1)
                    if p == 0 and blk == 2:
                        consts[1] = load_pass_consts(1)

                    def xslice(ko, lo, hi):
                        return xts[ko // 4][:, ko % 4, lo:hi]

                    # ---- projections: qT, kT, gateT ([dcol, tok]); v ([tok, dcol])
                    qsb = qkpool.tile([P, 2, BLK], F16, tag="qsb")
                    ksb = qkpool.tile([P, 2, BLK], F16, tag="ksb")
                    gsb = qkpool.tile([P, 2, BLK], F32, tag="gsb", bufs=1)
                    vsb = qkpool.tile([P, 4, 256], F16, tag="vsb")
                    for wsrc, dst, fn in ((wq_r, qsb, AF.Silu),
                                          (wk_r, ksb, AF.Silu)):
                        for m in range(2):
                            ps_ = pproj.tile([P, BLK], F32, tag="proj")
                            for ko in range(KO):
                                nc.tensor.matmul(
                                    ps_[:], wsrc[:, ko, m * P:(m + 1) * P],
                                    xslice(ko, 0, BLK),
                                    start=(ko == 0), stop=(ko == KO - 1))
                            nc.scalar.activation(dst[:, m, :], ps_[:], fn)
                    for m in range(4):  # v, token-quarter m
                        ps_ = pproj.tile([P, 256], F32, tag="proj")
                        for ko in range(KO):
                            nc.tensor.matmul(
                                ps_[:], xslice(ko, m * P, (m + 1) * P),
                                wv_r[:, ko, :],
                                start=(ko == 0), stop=(ko == KO - 1))
                        nc.scalar.activation(vsb[:, m, :], ps_[:], AF.Silu)
                    for m in range(2):  # gate last (single sigmoid table load)
                        ps_ = pproj.tile([P, BLK], F32, tag="proj")
                        for ko in range(KO):
                            nc.tensor.matmul(
                                ps_[:], wg_r[:, ko, m * P:(m + 1) * P],
                                xslice(ko, 0, BLK),
                                start=(ko == 0), stop=(ko == KO - 1))
                        nc.scalar.activation(gsb[:, m, :], ps_[:], AF.Sigmoid)

                    # ---- attention: 2 chunks of 256 per block
                    go_sb = apool.tile([P, 2, BLK], F16, tag="go")
                    for ch in range(2):
                        co = ch * C
                        first_chunk = (blk == 0 and ch == 0)
                        for lh in range(2):
                            # k * k_decay, transposed to [j, d] for the kv update
                            kdk = apool.tile([P, C], F16, tag="kdk", bufs=1)
                            nc.vector.tensor_tensor(kdk[:], ksb[:, lh, co:co + C],
                                                    kd_sb[:, lh, :], MUL)
                            kn_sb = apool.tile([P, 2, P], F16, tag="kn")
                            for jt in range(2):
                                knp = ptr.tile([P, P], F16, tag="tr")
                                nc.tensor.transpose(
                                    knp[:], kdk[:, jt * P:(jt + 1) * P], ident[:])
                                nc.vector.tensor_copy(kn_sb[:, jt, :], knp[:])
                            # scoresT[j, i] = (k q^T) * decayT
                            sm = apool.tile([P, 2, C], F16, tag="sm")
                            for jt in range(2):
                                st = psc.tile([P, C], F32, tag="sc")
                                nc.tensor.matmul(
                                    st[:], ksb[:, lh, co + jt * P:co + (jt + 1) * P],
                                    qsb[:, lh, co:co + C], start=True, stop=True)
                                nc.vector.tensor_tensor(sm[:, jt, :], st[:],
                                                        dd_sb[:, lh, jt, :], MUL)
                            qdq = apool.tile([P, C], F16, tag="qdq", bufs=1)
                            nc.vector.tensor_tensor(qdq[:], qsb[:, lh, co:co + C],
                                                    qd_sb[:, lh, :], MUL)
                            # oT[e, i] = v^T scoresT + kv^T (q * q_decay)
                            ot = psc.tile([P, C], F32, tag="sc")
                            nc.tensor.matmul(ot[:],
                                             vsb[:, 2 * ch, lh * P:(lh + 1) * P],
                                             sm[:, 0, :], start=True, stop=False)
                            nc.tensor.matmul(ot[:],
                                             vsb[:, 2 * ch + 1, lh * P:(lh + 1) * P],
                                             sm[:, 1, :], start=False,
                                             stop=first_chunk)
                            if not first_chunk:
                                kvr = apool.tile([P, P], F16, tag="kvr", bufs=1)
                                nc.vector.tensor_copy(kvr[:], kv_sb[:, lh, :])
                                nc.tensor.matmul(ot[:], kvr[:], qdq[:],
                                                 start=False, stop=True)
                            nc.vector.tensor_tensor(go_sb[:, lh, co:co + C], ot[:],
                                                    gsb[:, lh, co:co + C], MUL)
                            # kv <- bdecay * kv + (k kdecay)^T v
                            up = ptr.tile([P, P], F32, tag="tr")
                            for jt in range(2):
                                nc.tensor.matmul(up[:], kn_sb[:, jt, :],
                                                 vsb[:, 2 * ch + jt,
                                                     lh * P:(lh + 1) * P],
                                                 start=(jt == 0), stop=(jt == 1))
                            gh = 2 * p + lh
                            nc.vector.scalar_tensor_tensor(
                                kv_sb[:, lh, :], kv_sb[:, lh, :],
                                bd_sb[:, gh:gh + 1], up[:], MUL, ADD)

                    # ---- out projection (partial over this pass's 2 heads)
                    for mt in range(4):
                        for nt in range(4):
                            op = pout.tile([P, 512], F32, tag="out")
                            for lh in range(2):
                                nc.tensor.matmul(
                                    op[:], go_sb[:, lh, mt * P:(mt + 1) * P],
                                    wo_r[:, lh, nt * 512:(nt + 1) * 512],
                                    start=(lh == 0), stop=(lh == 1))
                            ob = opool.tile([P, 512], F32, tag="ob")
                            nc.vector.tensor_copy(ob[:], op[:])
                            nc.sync.dma_start(
                                outs[p].ap()[t0 + mt * P:t0 + (mt + 1) * P,
                                             nt * 512:(nt + 1) * 512],
                                ob[:])

    nc.compile()
    return nc


def _get_program():
    global _PROG
    if _PROG is None:
        _PROG = _build_program()
    return _PROG


def _prep_core_inputs(x, w_qkv, w_gate, w_out, slopes, core):
    b, q = core // 4, core % 4
    h0 = 4 * q
    s = np.asarray(slopes, dtype=np.float32).reshape(H)[h0:h0 + 4]  # [4]

    # x[b].T is [HID, S]; tile to [blk, quarter, kp, ko', t]
    xT = np.ascontiguousarray(
        x[b].T.reshape(4, 4, P, NBLK, BLK).transpose(3, 0, 2, 1, 4)
        .astype(np.float16))

    def wtile(w2):
        # [HID, 512] -> [pass, kp, ko, m=256]
        return np.ascontiguousarray(
            w2.reshape(KO, P, 2, 256).transpose(2, 1, 0, 3).astype(np.float16))

    cq = slice(h0 * D, h0 * D + 512)
    wq_c = wtile(w_qkv[:, cq])
    wk_c = wtile(w_qkv[:, 2048 + h0 * D: 2048 + h0 * D + 512])
    wv_c = wtile(w_qkv[:, 4096 + h0 * D: 4096 + h0 * D + 512])
    wg_c = wtile(w_gate[:, cq])
    # [512, HID] -> [pass, kp, kh, n]
    wo_c = np.ascontiguousarray(
        w_out[cq, :].reshape(2, 2, P, HID).transpose(0, 2, 1, 3)
        .astype(np.float16))

    pos = np.arange(C, dtype=np.float32)
    idx = pos[:, None] - pos[None, :]                      # [i, j] -> i - j
    ddT = np.empty((4, 2, P, C), dtype=np.float32)
    qdec = np.empty((4, P, C), dtype=np.float16)
    kdec = np.empty((4, P, C), dtype=np.float16)
    bdec = np.empty((P, 4), dtype=np.float32)
    for lh in range(4):
        sh = np.float64(s[lh])
        m = np.where(idx >= 0, np.exp(-sh * idx), 0.0)     # [i, j]
        ddT[lh] = m.T.reshape(2, P, C).astype(np.float32)  # [j, i] tiled
        qdec[lh] = np.broadcast_to(
            np.exp(-sh * (pos + 1.0)).astype(np.float16)[None, :], (P, C))
        kdec[lh] = np.broadcast_to(
            np.exp(-sh * (C - 1.0 - pos)).astype(np.float16)[None, :], (P, C))
        bdec[:, lh] = np.float32(np.exp(-sh * C))

    return {
        "xT": xT, "wq": wq_c, "wk": wk_c, "wv": wv_c, "wg": wg_c, "wo": wo_c,
        "ddT": ddT, "qdec": qdec, "kdec": kdec,
        "bdec": np.ascontiguousarray(bdec),
        "idn": np.eye(P, dtype=np.float16),
    }


def kernel(x, w_qkv, w_gate, w_out, slopes, _trace=False, _result_holder=None):
    from concourse.bass_utils import run_bass_kernel_spmd

    x = np.asarray(x, dtype=np.float32)
    w_qkv = np.asarray(w_qkv, dtype=np.float32)
    w_gate = np.asarray(w_gate, dtype=np.float32)
    w_out = np.asarray(w_out, dtype=np.float32)

    nc = _get_program()
    in_maps = [_prep_core_inputs(x, w_qkv, w_gate, w_out, slopes, c)
               for c in range(NCORES)]
    res = run_bass_kernel_spmd(nc, in_maps, core_ids=list(range(NCORES)),
                               trace=_trace)
    if _result_holder is not None:
        _result_holder.append(res)

    out = np.zeros((B, S, HID), dtype=np.float32)
    for c in range(NCORES):
        b = c // 4
        out[b] += res.results[c]["out0"]
        out[b] += res.results[c]["out1"]
    return out



# revision 6
# speedup vs baseline: 1.4326x; 1.4326x over previous
"""MiniMaxText01 linear attention on 8 trn2 NeuronCores — mixed fp16/fp8.

Sharding: core c -> batch b = c//4, head-quad q = c%4 (4 heads per core,
single merged pass). Row-parallel out_proj emits one fp16 partial
[S, HID] per core; the host sums 4 partials per batch.

Precision (chosen from measured per-stage error/speed tradeoffs;
fp8e4m3 DoubleRow = 2x tensor throughput on TRN2):
  - q/k projections + out_proj: fp16 (error-critical paths)
  - v/gate projections: fp8 DoubleRow (errors attenuated downstream)
  - attention intra-chunk o and kv-update: fp8 DoubleRow over the
    256-token contraction; scores and kv*q stay fp16.
"""

import numpy as np
import ml_dtypes

B, S, HID = 2, 4096, 2048
H, D, C = 16, 128, 256
BLK = 512              # token block (2 chunks)
NBLK = S // BLK        # 8
KO = HID // 128        # 16 fp16 contraction subtiles
KO2 = HID // 256       # 8 fp8 DoubleRow slab-pairs
NCORES = 8
P = 128

E4NP = ml_dtypes.float8_e4m3   # HW float8e4 semantics (max 240)

_PROG = None


def _build_program():
    import concourse.bacc as bacc
    import concourse.mybir as mybir
    import concourse.tile as tile

    F32 = mybir.dt.float32
    F16 = mybir.dt.float16
    F8 = mybir.dt.float8e4
    AF = mybir.ActivationFunctionType
    MUL = mybir.AluOpType.mult
    ADD = mybir.AluOpType.add
    DR = mybir.MatmulPerfMode.DoubleRow

    nc = bacc.Bacc("TRN2", target_bir_lowering=False, debug=False,
                   num_devices=NCORES)

    x16_d = nc.dram_tensor("x16", [NBLK, P, KO, BLK], F16,
                           kind="ExternalInput")
    x8_d = nc.dram_tensor("x8", [NBLK, P, KO2, 2, BLK], F8,
                          kind="ExternalInput")
    wq_d = nc.dram_tensor("wq", [P, KO, 512], F16, kind="ExternalInput")
    wk_d = nc.dram_tensor("wk", [P, KO, 512], F16, kind="ExternalInput")
    wv_d = nc.dram_tensor("wv", [P, KO2, 2, 512], F8, kind="ExternalInput")
    wg_d = nc.dram_tensor("wg", [P, KO2, 2, 512], F8, kind="ExternalInput")
    wo_d = nc.dram_tensor("wo", [P, 4, HID], F16, kind="ExternalInput")
    ddT_d = nc.dram_tensor("ddT", [4, 2, P, C], F16, kind="ExternalInput")
    qdec_d = nc.dram_tensor("qdec", [4, P, C], F16, kind="ExternalInput")
    kdecT_d = nc.dram_tensor("kdecT", [P, 8], F32, kind="ExternalInput")
    bdec_d = nc.dram_tensor("bdec", [P, 4], F32, kind="ExternalInput")
    idn_d = nc.dram_tensor("idn", [P, P], F16, kind="ExternalInput")
    outd = nc.dram_tensor("out", [S, HID], F16, kind="ExternalOutput")

    with tile.TileContext(nc) as tc:
        with tc.tile_pool(name="const", bufs=1) as cpool, \
             tc.tile_pool(name="wpool", bufs=1) as wpool, \
             tc.tile_pool(name="xpool", bufs=2) as xpool, \
             tc.tile_pool(name="qkpool", bufs=2) as qkpool, \
             tc.tile_pool(name="apool", bufs=2) as apool, \
             tc.tile_pool(name="kvpool", bufs=1) as kvpool, \
             tc.tile_pool(name="opool", bufs=3) as opool, \
             tc.tile_pool(name="pproj", bufs=2, space="PSUM") as pproj, \
             tc.tile_pool(name="pattn", bufs=3, space="PSUM") as pattn, \
             tc.tile_pool(name="ptr", bufs=1, space="PSUM") as ptr, \
             tc.tile_pool(name="pout", bufs=2, space="PSUM") as pout:

            def load_x(blk):
                x16 = xpool.tile([P, KO, BLK], F16, tag="x16", name="x16")
                nc.sync.dma_start(x16[:], x16_d.ap()[blk])
                x8 = xpool.tile([P, KO2, 2, BLK], F8, tag="x8", name="x8")
                nc.sync.dma_start(x8[:], x8_d.ap()[blk])
                return x16, x8

            cur_x = load_x(0)

            wq_s = wpool.tile([P, KO, 512], F16, tag="wq")
            nc.sync.dma_start(wq_s[:], wq_d.ap())
            wk_s = wpool.tile([P, KO, 512], F16, tag="wk")
            nc.sync.dma_start(wk_s[:], wk_d.ap())
            wv_s = wpool.tile([P, KO2, 2, 512], F8, tag="wv")
            nc.sync.dma_start(wv_s[:], wv_d.ap())
            wg_s = wpool.tile([P, KO2, 2, 512], F8, tag="wg")
            nc.sync.dma_start(wg_s[:], wg_d.ap())
            wo_s = wpool.tile([P, 4, HID], F16, tag="wo")
            nc.sync.dma_start(wo_s[:], wo_d.ap())

            dd_sb = cpool.tile([P, 4, 2, C], F16)
            qd_sb = cpool.tile([P, 4, C], F16)
            for lh in range(4):
                for jt in range(2):
                    nc.sync.dma_start(dd_sb[:, lh, jt, :], ddT_d.ap()[lh, jt])
                nc.sync.dma_start(qd_sb[:, lh, :], qdec_d.ap()[lh])
            kdT = cpool.tile([P, 8], F32)
            nc.sync.dma_start(kdT[:], kdecT_d.ap())
            bd_sb = cpool.tile([P, 4], F32)
            nc.sync.dma_start(bd_sb[:], bdec_d.ap())
            ident = cpool.tile([P, P], F16)
            nc.sync.dma_start(ident[:], idn_d.ap())

            kv_sb = kvpool.tile([P, 4, P], F32, tag="kv")
            nc.vector.memset(kv_sb[:], 0.0)

            for blk in range(NBLK):
                t0 = blk * BLK
                x16_t, x8_t = cur_x
                if blk + 1 < NBLK:
                    cur_x = load_x(blk + 1)

                qsb = qkpool.tile([P, 4, BLK], F16, tag="qsb")
                ksb = qkpool.tile([P, 4, BLK], F16, tag="ksb")
                gsb = qkpool.tile([P, 4, BLK], F16, tag="gsb")
                vsb = qkpool.tile([P, 4, BLK], F8, tag="vsb")

                # q/k projections: fp16
                for wt, dst in ((wq_s, qsb), (wk_s, ksb)):
                    for m in range(4):
                        ps = pproj.tile([P, BLK], F32, tag="proj")
                        for ko in range(KO):
                            nc.tensor.matmul(ps[:],
                                             wt[:, ko, m * P:(m + 1) * P],
                                             x16_t[:, ko, :],
                                             start=(ko == 0),
                                             stop=(ko == KO - 1))
                        nc.scalar.activation(dst[:, m, :], ps[:], AF.Silu)
                # v projection: fp8 DR (lhsT = x8 token tile)
                for m in range(4):
                    ps = pproj.tile([P, BLK], F32, tag="proj")
                    for kk in range(KO2):
                        nc.tensor.matmul(ps[:],
                                         x8_t[:, kk, :, m * P:(m + 1) * P],
                                         wv_s[:, kk, :, :],
                                         start=(kk == 0),
                                         stop=(kk == KO2 - 1), perf_mode=DR)
                    nc.scalar.activation(vsb[:, m, :], ps[:], AF.Silu,
                                         scale=1.0 / 64.0)
                # gate projection: fp8 DR (last: one sigmoid table load)
                for m in range(4):
                    ps = pproj.tile([P, BLK], F32, tag="proj")
                    for kk in range(KO2):
                        nc.tensor.matmul(ps[:],
                                         wg_s[:, kk, :, m * P:(m + 1) * P],
                                         x8_t[:, kk, :, :],
                                         start=(kk == 0),
                                         stop=(kk == KO2 - 1), perf_mode=DR)
                    nc.scalar.activation(gsb[:, m, :], ps[:], AF.Sigmoid,
                                         scale=1.0 / 64.0)

                # ---- attention: 2 chunks of 256
                go = qkpool.tile([P, 4, BLK], F16, tag="go")
                for ch in range(2):
                    co = ch * C
                    first_chunk = (blk == 0 and ch == 0)
                    for lh in range(4):
                        sm = apool.tile([P, 2, C], F8, tag="sm")
                        for jt in range(2):
                            st = pattn.tile([P, C], F32, tag="sc")
                            nc.tensor.matmul(
                                st[:],
                                ksb[:, lh, co + jt * P:co + (jt + 1) * P],
                                qsb[:, lh, co:co + C], start=True, stop=True)
                            nc.vector.tensor_tensor(sm[:, jt, :], st[:],
                                                    dd_sb[:, lh, jt, :], MUL)
                        ot = pattn.tile([P, C], F32, tag="sc")
                        nc.tensor.matmul(ot[:],
                                         vsb[:, 2 * ch:2 * ch + 2,
                                             lh * P:(lh + 1) * P],
                                         sm[:, :, :], start=True,
                                         stop=first_chunk, perf_mode=DR)
                        if not first_chunk:
                            qdq = apool.tile([P, C], F16, tag="qdq")
                            nc.vector.tensor_tensor(qdq[:],
                                                    qsb[:, lh, co:co + C],
                                                    qd_sb[:, lh, :], MUL)
                            kvr = apool.tile([P, P], F16, tag="kvr")
                            nc.vector.tensor_copy(kvr[:], kv_sb[:, lh, :])
                            nc.tensor.matmul(ot[:], kvr[:], qdq[:],
                                             start=False, stop=True)
                        nc.vector.tensor_tensor(go[:, lh, co:co + C], ot[:],
                                                gsb[:, lh, co:co + C], MUL)
                        # kv <- bdecay*kv + (k*kdec)^T v
                        kn = apool.tile([P, 2, P], F8, tag="kn")
                        for jt in range(2):
                            tp = ptr.tile([P, P], F16, tag="tr")
                            nc.tensor.transpose(
                                tp[:],
                                ksb[:, lh, co + jt * P:co + (jt + 1) * P],
                                ident[:])
                            ci = lh * 2 + jt
                            nc.vector.tensor_scalar(kn[:, jt, :], tp[:],
                                                    kdT[:, ci:ci + 1], None,
                                                    MUL)
                        up = pattn.tile([P, C], F32, tag="sc", name="up")
                        nc.tensor.matmul(up[:, :P], kn[:, :, :],
                                         vsb[:, 2 * ch:2 * ch + 2,
                                             lh * P:(lh + 1) * P],
                                         start=True, stop=True, perf_mode=DR)
                        nc.vector.scalar_tensor_tensor(
                            kv_sb[:, lh, :], kv_sb[:, lh, :],
                            bd_sb[:, lh:lh + 1], up[:, :P], MUL, ADD)

                # ---- out projection: fp16 (row-parallel partial)
                for mt in range(4):
                    ob = opool.tile([P, 4, 512], F16, tag="ob")
                    for nt in range(4):
                        po = pout.tile([P, 512], F32, tag="out")
                        for kh in range(4):
                            nc.tensor.matmul(
                                po[:], go[:, kh, mt * P:(mt + 1) * P],
                                wo_s[:, kh, nt * 512:(nt + 1) * 512],
                                start=(kh == 0), stop=(kh == 3))
                        nc.scalar.copy(ob[:, nt, :], po[:])
                    nc.sync.dma_start(
                        outd.ap()[t0 + mt * P:t0 + (mt + 1) * P, :], ob[:])

    nc.compile()
    return nc


def _get_program():
    global _PROG
    if _PROG is None:
        _PROG = _build_program()
    return _PROG


_XCACHE = {}


def _prep_x(x, b):
    if b not in _XCACHE:
        xT = np.ascontiguousarray(x[b].T.astype(np.float32))     # [HID, S]
        x16 = np.ascontiguousarray(
            xT.reshape(KO, P, NBLK, BLK).transpose(2, 1, 0, 3)
        ).astype(np.float16)
        x8 = np.ascontiguousarray(
            xT.reshape(KO2, 2, P, NBLK, BLK).transpose(3, 2, 0, 1, 4)
        ).astype(E4NP)
        _XCACHE[b] = {"x16": x16, "x8": x8}
    return _XCACHE[b]


def _prep_core_inputs(x, w_qkv, w_gate, w_out, slopes, core):
    b, q = core // 4, core % 4
    h0 = 4 * q
    s = np.asarray(slopes, dtype=np.float64).reshape(H)[h0:h0 + 4]

    d = dict(_prep_x(x, b))

    def wtile16(w2):
        # [HID, 512] -> [P, KO, 512]
        return np.ascontiguousarray(
            w2.astype(np.float32).reshape(KO, P, 512).transpose(1, 0, 2)
        ).astype(np.float16)

    def wtile8(w2):
        # [HID, 512] -> [P, KO2, 2, 512], scaled x64
        return np.ascontiguousarray(
            (w2.astype(np.float32) * 64.0)
            .reshape(KO2, 2, P, 512).transpose(2, 0, 1, 3)).astype(E4NP)

    cq = slice(h0 * D, h0 * D + 512)
    d["wq"] = wtile16(w_qkv[:, cq])
    d["wk"] = wtile16(w_qkv[:, 2048 + h0 * D:2048 + h0 * D + 512])
    d["wv"] = wtile8(w_qkv[:, 4096 + h0 * D:4096 + h0 * D + 512])
    d["wg"] = wtile8(w_gate[:, cq])
    # [512, HID] -> [P, 4, HID]
    d["wo"] = np.ascontiguousarray(
        w_out[cq, :].astype(np.float32).reshape(4, P, HID).transpose(1, 0, 2)
    ).astype(np.float16)

    pos = np.arange(C, dtype=np.float64)
    idx = pos[:, None] - pos[None, :]                     # i - j
    ddT = np.empty((4, 2, P, C), dtype=np.float16)
    qdec = np.empty((4, P, C), dtype=np.float16)
    kdecT = np.empty((P, 8), dtype=np.float32)
    bdec = np.empty((P, 4), dtype=np.float32)
    for lh in range(4):
        sh = s[lh]
        m = np.where(idx >= 0, np.exp(-sh * idx), 0.0)    # [i, j]
        ddT[lh] = m.T.reshape(2, P, C).astype(np.float16)
        qdec[lh] = np.broadcast_to(
            np.exp(-sh * (pos + 1.0)).astype(np.float16)[None, :], (P, C))
        for jt in range(2):
            jj = jt * P + np.arange(P, dtype=np.float64)
            kdecT[:, lh * 2 + jt] = np.exp(-sh * (C - 1.0 - jj))
        bdec[:, lh] = np.float32(np.exp(-sh * C))

    d.update(ddT=ddT, qdec=qdec, kdecT=kdecT,
             bdec=np.ascontiguousarray(bdec),
             idn=np.eye(P, dtype=np.float16))
    return d


def kernel(x, w_qkv, w_gate, w_out, slopes, _trace=False, _result_holder=None):
    from concourse.bass_utils import run_bass_kernel_spmd

    x = np.asarray(x, dtype=np.float32)
    w_qkv = np.asarray(w_qkv, dtype=np.float32)
    w_gate = np.asarray(w_gate, dtype=np.float32)
    w_out = np.asarray(w_out, dtype=np.float32)

    _XCACHE.clear()
    nc = _get_program()
    in_maps = [_prep_core_inputs(x, w_qkv, w_gate, w_out, slopes, c)
               for c in range(NCORES)]
    _XCACHE.clear()
    res = run_bass_kernel_spmd(nc, in_maps, core_ids=list(range(NCORES)),
                               trace=_trace)
    if _result_holder is not None:
        _result_holder.append(res)

    out = np.zeros((B, S, HID), dtype=np.float32)
    for c in range(NCORES):
        out[c // 4] += res.results[c]["out"].astype(np.float32)
    return out
